# revision 14
# baseline (speedup 1.0000x reference)
"""Causal multi-head attention (B=2, T=2048, DIM=2048, H=16, HD=128) on 8
Trainium2 NeuronCores.

Sharding: core = 4*b + g  (b = batch 0..1, g = head-group 0..3, 4 heads each).
Each core computes, for its batch b and heads 4g..4g+3:
  QKV projection -> causal attention -> partial out = attn_out @ wo[rows of g]
The host sums the 4 partial outputs per batch (the "all-reduce after wo").

On-device layout avoids every transpose:
  - host passes x[b].T, so projections contract d with d on partitions
  - Q^T/K^T kept as [hd, t] (head dim on partitions)
  - scores computed as S^T = K^T_tile.T @ Q^T  ([j, i] layout)
  - exp via ScalarE; causal masking = multiply diagonal tiles by 0/1 masks
  - P@V computed as O^T via lhsT = V tile (natural [t, hd] layout)
  - denominator via ones-vector matmul, normalization via GPSIMD
    partition-broadcast of 1/d + VectorE multiply
  - wo projection consumes O^T tiles directly as stationary operands
Matmuls run as float32r (full PE rate; ~2-7e-4 rel err end to end).
"""

import math
import os

import numpy as np

B, T, D, H, HD = 2, 2048, 2048, 16, 128
NH = 4            # heads per core
NCORES = 8
TCH = 512         # query-chunk width (moving-operand free size)
NDT = D // 128    # 16 d-tiles (contraction tiles for projections)
NTT = T // 128    # 16 t-tiles
NCH = T // TCH    # 4 query chunks
DQ = 4            # d-tiles per accumulation group (PSUM chain length)
NQ = NDT // DQ    # 4 groups

_BUILT = {}
LAST_RESULTS = None  # BassKernelResults of the most recent kernel() call


def _build(causal: bool):
    import concourse.mybir as mybir
    import concourse.tile as tile
    from concourse import bacc

    F32 = mybir.dt.float32
    F32R = mybir.dt.float32r
    EXP = mybir.ActivationFunctionType.Exp
    scale = 1.0 / math.sqrt(HD)

    nc = bacc.Bacc(None, name="attn")
    xT = nc.dram_tensor("xT", [D, T], F32R, kind="ExternalInput")
    wqkv = nc.dram_tensor("wqkv", [D, 3 * NH * HD], F32R, kind="ExternalInput")
    wo = nc.dram_tensor("wo", [NH * HD, D], F32R, kind="ExternalInput")
    masks = nc.dram_tensor("masks", [4, 128, TCH], F32R, kind="ExternalInput")
    if not causal:
        maskT = nc.dram_tensor("maskT", [T, T], F32R, kind="ExternalInput")
    out = nc.dram_tensor("out", [T, D], F32, kind="ExternalOutput")

    with tile.TileContext(nc) as tc:
        with (
            tc.tile_pool(name="persist", bufs=1) as persist,
            tc.tile_pool(name="ps4", bufs=4, space="PSUM") as ps4,
            tc.tile_pool(name="ps2", bufs=2, space="PSUM") as ps2,
        ):
            # persistent operands for the attention phase
            qkt = persist.tile([128, 8, T], F32R)          # slots 0-3: Q^T heads, 4-7: K^T heads
            vsb = persist.tile([128, NTT, NH * HD], F32R)  # V, [t-tile][local t, head*hd]
            msb = persist.tile([128, 4, TCH], F32R)   # diagonal causal masks
            nc.sync.dma_start(msb[:], masks.rearrange("q p i -> p q i"))
            ones_f = persist.tile([128, 1], F32)
            ones = persist.tile([128, 1], F32R)
            nc.vector.memset(ones_f[:], 1.0)
            nc.vector.tensor_copy(ones[:], ones_f[:])

            # ---- Phase A: QKV projections, streaming x^T / wqkv d-tiles ----
            with tc.tile_pool(name="xw", bufs=7) as xw:
                for qg in range(NQ):
                    xts, wqks, wvs = [], [], []
                    for k in range(DQ):
                        di = qg * DQ + k
                        xt_t = xw.tile([128, T], F32R, tag="xt")
                        nc.sync.dma_start(xt_t[:], xT[di * 128:(di + 1) * 128, :])
                        wqk_t = xw.tile([128, 2 * NH * HD], F32R, tag="wqk")
                        nc.sync.dma_start(wqk_t[:],
                                          wqkv[di * 128:(di + 1) * 128, 0:2 * NH * HD])
                        wv_t = xw.tile([128, NH * HD], F32R, tag="wv")
                        nc.sync.dma_start(wv_t[:],
                                          wqkv[di * 128:(di + 1) * 128,
                                               2 * NH * HD:3 * NH * HD])
                        xts.append(xt_t)
                        wqks.append(wqk_t)
                        wvs.append(wv_t)
                    # V first: attention's PV chains need V earliest
                    for tt in range(NTT):
                        ps = ps4.tile([128, NH * HD], F32, tag="ps_s")
                        for k in range(DQ):
                            nc.tensor.matmul(
                                ps[:],
                                xts[k][:, tt * 128:(tt + 1) * 128],
                                wvs[k][:],
                                start=(k == 0),
                                stop=(k == DQ - 1),
                            )
                        dst = vsb[:, tt, :]
                        if qg == 0:
                            nc.scalar.copy(dst, ps[:])
                        else:
                            nc.vector.tensor_add(dst, dst, ps[:])
                    # Q^T / K^T, ordered so K0/Q0 chunk 0 completes first
                    for tch in range(NCH):
                        for s in (4, 0, 5, 1, 6, 2, 7, 3):
                            ps = ps4.tile([128, TCH], F32, tag="ps_s")
                            for k in range(DQ):
                                nc.tensor.matmul(
                                    ps[:],
                                    wqks[k][:, s * 128:(s + 1) * 128],
                                    xts[k][:, tch * TCH:(tch + 1) * TCH],
                                    start=(k == 0),
                                    stop=(k == DQ - 1),
                                )
                            dst = qkt[:, s, tch * TCH:(tch + 1) * TCH]
                            if qg == 0:
                                nc.scalar.copy(dst, ps[:])
                            else:
                                nc.vector.tensor_add(dst, dst, ps[:])

            # Phase B/C pools open after the stream pool closed, reusing its
            # SBUF region. wo loads land in that freed space.
            with (
                tc.tile_pool(name="post", bufs=1) as post,
                tc.tile_pool(name="work", bufs=4) as work,
                tc.tile_pool(name="sml", bufs=2) as sml,
                tc.tile_pool(name="otp", bufs=2) as otp,
                tc.tile_pool(name="outp", bufs=2) as outp,
            ):
                # separate 2D tiles: a DMA into a 3D middle-index slice
                # ([128, 1, D] dst AP) hard-faults the exec unit
                wosb = []
                for et in range(NH):
                    wt_ = post.tile([128, D], F32R, tag=f"wos{et}")
                    nc.sync.dma_start(wt_[:], wo[et * 128:(et + 1) * 128, :])
                    wosb.append(wt_)

                # ---- Phase B+C: attention per chunk, then wo projection ----
                for c in range(NCH):
                    otc = otp.tile([128, NH, TCH], F32R, tag="ot")
                    njt = 4 * (c + 1) if causal else NTT
                    for h in range(NH):
                        pso = ps2.tile([128, TCH], F32, tag="ps_o")
                        psd = ps2.tile([1, TCH], F32, tag="ps_d")

                        def emit_pss(jt):
                            pss = ps4.tile([128, TCH], F32, tag="ps_s")
                            nc.tensor.matmul(
                                pss[:],
                                qkt[:, 4 + h, jt * 128:(jt + 1) * 128],
                                qkt[:, h, c * TCH:(c + 1) * TCH],
                                start=True,
                                stop=True,
                            )
                            pt = work.tile([128, TCH], F32R, tag="pt")
                            nc.scalar.activation(pt[:], pss[:], EXP, scale=scale)
                            if causal:
                                qd = jt - 4 * c
                                if qd >= 0:
                                    # GpSimd, not DVE: keeps the masking off the
                                    # DVE FIFO (which drains combine-adds at the
                                    # phase boundary) at ~2x the op cost on an
                                    # otherwise idle engine
                                    nc.gpsimd.tensor_mul(pt[:], pt[:], msb[:, qd, :])
                            else:
                                mt = work.tile([128, TCH], F32R, tag="mt")
                                nc.sync.dma_start(
                                    mt[:],
                                    maskT[jt * 128:(jt + 1) * 128,
                                          c * TCH:(c + 1) * TCH],
                                )
                                nc.vector.tensor_mul(pt[:], pt[:], mt[:])
                            return pt

                        # run the S^T matmul + exp two tiles ahead so the
                        # in-order PE never waits on ScalarE's exp
                        pts = {}
                        for jt in range(min(2, njt)):
                            pts[jt] = emit_pss(jt)
                        for jt in range(njt):
                            if jt + 2 < njt:
                                pts[jt + 2] = emit_pss(jt + 2)
                            pt = pts.pop(jt)
                            nc.tensor.matmul(
                                pso[:],
                                vsb[:, jt, h * HD:(h + 1) * HD],
                                pt[:],
                                start=(jt == 0),
                                stop=(jt == njt - 1),
                            )
                            nc.tensor.matmul(
                                psd[:],
                                ones[:, 0:1],
                                pt[:],
                                start=(jt == 0),
                                stop=(jt == njt - 1),
                            )
                        # 1/d via single-op approx reciprocal (~18 bits, way
                        # beyond the ~12-bit fp32r pipeline); exact reciprocal
                        # costs 3.3us and Ln/Exp thrash the ACT table
                        drc = sml.tile([1, TCH], F32, tag="drc")
                        nc.vector.reciprocal_approx_fast(drc[:], psd[:])
                        bc = sml.tile([128, TCH], F32, tag="bc")
                        nc.gpsimd.partition_broadcast(bc[:], drc[:])
                        nc.vector.tensor_mul(otc[:, h, :], pso[:], bc[:])
                    # wo projection for the 4 t-tiles of this chunk
                    for lt in range(4):
                        for oc in range(NCH):
                            ps = ps2.tile([128, TCH], F32, tag="ps_o")
                            for h2 in range(NH):
                                nc.tensor.matmul(
                                    ps[:],
                                    otc[:, h2, lt * 128:(lt + 1) * 128],
                                    wosb[h2][:, oc * TCH:(oc + 1) * TCH],
                                    start=(h2 == 0),
                                    stop=(h2 == NH - 1),
                                )
                            ost = outp.tile([128, TCH], F32, tag="ost")
                            nc.vector.tensor_copy(ost[:], ps[:])
                            nc.sync.dma_start(
                                out[(4 * c + lt) * 128:(4 * c + lt + 1) * 128,
                                    oc * TCH:(oc + 1) * TCH],
                                ost[:],
                            )
    nc.compile()
    return nc


def _get_built(causal: bool):
    if causal not in _BUILT:
        _BUILT[causal] = _build(causal)
    return _BUILT[causal]


def _diag_masks():
    # masks[q][jl, ii] = 1 if key (128*q + jl) <= query ii within the chunk
    q = np.arange(4)[:, None, None]
    jl = np.arange(128)[None, :, None]
    ii = np.arange(TCH)[None, None, :]
    return (ii >= 128 * q + jl).astype(np.float32)


def kernel(x, mask, wqkv, wo):
    global LAST_RESULTS
    from concourse.bass_utils import run_bass_kernel_spmd

    x = np.ascontiguousarray(np.asarray(x, dtype=np.float32))
    wqkv = np.ascontiguousarray(np.asarray(wqkv, dtype=np.float32))
    wo_f = np.ascontiguousarray(np.asarray(wo, dtype=np.float32))
    mask_np = np.asarray(mask).reshape(T, T).astype(bool)
    causal = bool(np.array_equal(mask_np, np.tril(np.ones((T, T), dtype=bool))))

    nc = _get_built(causal)
    masks_arr = _diag_masks()
    maskT = None
    if not causal:
        maskT = np.ascontiguousarray(mask_np.T.astype(np.float32))

    in_maps = []
    for core in range(NCORES):
        b, g = divmod(core, NH)
        xT = np.ascontiguousarray(x[b].T)
        wq = wqkv[:, 0 * H * HD + g * NH * HD:0 * H * HD + (g + 1) * NH * HD]
        wk = wqkv[:, 1 * H * HD + g * NH * HD:1 * H * HD + (g + 1) * NH * HD]
        wv = wqkv[:, 2 * H * HD + g * NH * HD:2 * H * HD + (g + 1) * NH * HD]
        wqkv_g = np.ascontiguousarray(np.concatenate([wq, wk, wv], axis=1))
        wo_g = np.ascontiguousarray(wo_f[g * NH * HD:(g + 1) * NH * HD, :])
        m = {"xT": xT, "wqkv": wqkv_g, "wo": wo_g, "masks": masks_arr}
        if maskT is not None:
            m["maskT"] = maskT
        in_maps.append(m)

    trace = bool(os.environ.get("ATTN_TRACE"))
    res = run_bass_kernel_spmd(nc, in_maps, core_ids=list(range(NCORES)),
                               trace=trace)
    LAST_RESULTS = res

    acc = np.zeros((B, T, D), dtype=np.float64)
    for core in range(NCORES):
        b = core // NH
        acc[b] += res.results[core]["out"].astype(np.float64)
    return acc.astype(np.float32)


# revision 15
# speedup vs baseline: 1.3770x; 1.3770x over previous
"""Causal multi-head attention (B=2, T=2048, DIM=2048, H=16, HD=128) on 8
Trainium2 NeuronCores.

Sharding: core = 4*b + g  (b = batch 0..1, g = head-group 0..3, 4 heads each).
Each core computes, for its batch b and heads 4g..4g+3:
  QKV projection -> causal attention -> partial out = attn_out @ wo[rows of g]
The host sums the 4 partial outputs per batch (the "all-reduce after wo").

On-device layout avoids every transpose:
  - host passes x[b].T, so projections contract d with d on partitions
  - Q^T/K^T kept as [hd, t] (head dim on partitions)
  - scores computed as S^T = K^T_tile.T @ Q^T  ([j, i] layout)
  - exp via ScalarE; causal masking = multiply diagonal tiles by 0/1 masks
  - P@V computed as O^T via lhsT = V tile (natural [t, hd] layout)
  - denominator via ones-vector matmul, normalization via GPSIMD
    partition-broadcast of 1/d + VectorE multiply
  - wo projection consumes O^T tiles directly as stationary operands
Matmuls run as float32r (full PE rate; ~2-7e-4 rel err end to end).
"""

import math
import os

import numpy as np

B, T, D, H, HD = 2, 2048, 2048, 16, 128
NH = 4            # heads per core
NCORES = 8
TCH = 512         # query-chunk width (moving-operand free size)
NDT = D // 128    # 16 d-tiles (contraction tiles for projections)
NTT = T // 128    # 16 t-tiles
NCH = T // TCH    # 4 query chunks
DQ = 4            # d-tiles per accumulation group (PSUM chain length)
NQ = NDT // DQ    # 4 groups

_BUILT = {}
LAST_RESULTS = None  # BassKernelResults of the most recent kernel() call


def _build(causal: bool):
    import concourse.mybir as mybir
    import concourse.tile as tile
    from concourse import bacc

    F32 = mybir.dt.float32
    F32R = mybir.dt.float32r
    EXP = mybir.ActivationFunctionType.Exp
    scale = 1.0 / math.sqrt(HD)

    nc = bacc.Bacc(None, name="attn")
    xT = nc.dram_tensor("xT", [D, T], F32R, kind="ExternalInput")
    wqkv = nc.dram_tensor("wqkv", [D, 3 * NH * HD], F32R, kind="ExternalInput")
    wo = nc.dram_tensor("wo", [NH * HD, D], F32R, kind="ExternalInput")
    masks = nc.dram_tensor("masks", [4, 128, TCH], F32R, kind="ExternalInput")
    if not causal:
        maskT = nc.dram_tensor("maskT", [T, T], F32R, kind="ExternalInput")
    out = nc.dram_tensor("out", [T, D], F32, kind="ExternalOutput")

    with tile.TileContext(nc) as tc:
        with (
            tc.tile_pool(name="persist", bufs=1) as persist,
            tc.tile_pool(name="ps4", bufs=4, space="PSUM") as ps4,
            tc.tile_pool(name="ps2", bufs=2, space="PSUM") as ps2,
        ):
            # persistent operands for the attention phase
            qkt = persist.tile([128, 8, T], F32R)          # slots 0-3: Q^T heads, 4-7: K^T heads
            vsb = persist.tile([128, NTT, NH * HD], F32R)  # V, [t-tile][local t, head*hd]
            msb = persist.tile([128, 4, TCH], F32R)   # diagonal causal masks
            nc.sync.dma_start(msb[:], masks.rearrange("q p i -> p q i"))
            ones_f = persist.tile([128, 1], F32)
            ones = persist.tile([128, 1], F32R)
            nc.vector.memset(ones_f[:], 1.0)
            nc.vector.tensor_copy(ones[:], ones_f[:])

            # ---- Phase A: QKV projections, streaming x^T / wqkv d-tiles ----
            with tc.tile_pool(name="xw", bufs=7) as xw:
                for qg in range(NQ):
                    xts, wqks, wvs = [], [], []
                    for k in range(DQ):
                        di = qg * DQ + k
                        xt_t = xw.tile([128, T], F32R, tag="xt")
                        nc.sync.dma_start(xt_t[:], xT[di * 128:(di + 1) * 128, :])
                        wqk_t = xw.tile([128, 2 * NH * HD], F32R, tag="wqk")
                        nc.sync.dma_start(wqk_t[:],
                                          wqkv[di * 128:(di + 1) * 128, 0:2 * NH * HD])
                        wv_t = xw.tile([128, NH * HD], F32R, tag="wv")
                        nc.sync.dma_start(wv_t[:],
                                          wqkv[di * 128:(di + 1) * 128,
                                               2 * NH * HD:3 * NH * HD])
                        xts.append(xt_t)
                        wqks.append(wqk_t)
                        wvs.append(wv_t)
                    # V first: attention's PV chains need V earliest
                    for tt in range(NTT):
                        ps = ps4.tile([128, NH * HD], F32, tag="ps_s")
                        for k in range(DQ):
                            nc.tensor.matmul(
                                ps[:],
                                xts[k][:, tt * 128:(tt + 1) * 128],
                                wvs[k][:],
                                start=(k == 0),
                                stop=(k == DQ - 1),
                            )
                        dst = vsb[:, tt, :]
                        if qg == 0:
                            nc.scalar.copy(dst, ps[:])
                        else:
                            nc.vector.tensor_add(dst, dst, ps[:])
                    # Q^T / K^T, ordered so K0/Q0 chunk 0 completes first
                    for tch in range(NCH):
                        for s in (4, 0, 5, 1, 6, 2, 7, 3):
                            ps = ps4.tile([128, TCH], F32, tag="ps_s")
                            for k in range(DQ):
                                nc.tensor.matmul(
                                    ps[:],
                                    wqks[k][:, s * 128:(s + 1) * 128],
                                    xts[k][:, tch * TCH:(tch + 1) * TCH],
                                    start=(k == 0),
                                    stop=(k == DQ - 1),
                                )
                            dst = qkt[:, s, tch * TCH:(tch + 1) * TCH]
                            if qg == 0:
                                nc.scalar.copy(dst, ps[:])
                            else:
                                nc.vector.tensor_add(dst, dst, ps[:])

            # Phase B/C pools open after the stream pool closed, reusing its
            # SBUF region. wo loads land in that freed space.
            with (
                tc.tile_pool(name="post", bufs=1) as post,
                tc.tile_pool(name="work", bufs=4) as work,
                tc.tile_pool(name="sml", bufs=2) as sml,
                tc.tile_pool(name="otp", bufs=2) as otp,
                tc.tile_pool(name="outp", bufs=2) as outp,
            ):
                # separate 2D tiles: a DMA into a 3D middle-index slice
                # ([128, 1, D] dst AP) hard-faults the exec unit
                wosb = []
                for et in range(NH):
                    wt_ = post.tile([128, D], F32R, tag=f"wos{et}")
                    nc.sync.dma_start(wt_[:], wo[et * 128:(et + 1) * 128, :])
                    wosb.append(wt_)

                # ---- Phase B+C: attention per chunk, then wo projection ----
                for c in range(NCH):
                    otc = otp.tile([128, NH, TCH], F32R, tag="ot")
                    njt = 4 * (c + 1) if causal else NTT
                    for h in range(NH):
                        pso = ps2.tile([128, TCH], F32, tag="ps_o")
                        psd = ps2.tile([1, TCH], F32, tag="ps_d")

                        def emit_pss(jt):
                            pss = ps4.tile([128, TCH], F32, tag="ps_s")
                            nc.tensor.matmul(
                                pss[:],
                                qkt[:, 4 + h, jt * 128:(jt + 1) * 128],
                                qkt[:, h, c * TCH:(c + 1) * TCH],
                                start=True,
                                stop=True,
                            )
                            pt = work.tile([128, TCH], F32R, tag="pt")
                            nc.scalar.activation(pt[:], pss[:], EXP, scale=scale)
                            if causal:
                                qd = jt - 4 * c
                                if qd >= 0:
                                    nc.vector.tensor_mul(pt[:], pt[:], msb[:, qd, :])
                            else:
                                mt = work.tile([128, TCH], F32R, tag="mt")
                                nc.sync.dma_start(
                                    mt[:],
                                    maskT[jt * 128:(jt + 1) * 128,
                                          c * TCH:(c + 1) * TCH],
                                )
                                nc.vector.tensor_mul(pt[:], pt[:], mt[:])
                            return pt

                        # run the S^T matmul + exp two tiles ahead so the
                        # in-order PE never waits on ScalarE's exp
                        pts = {}
                        for jt in range(min(2, njt)):
                            pts[jt] = emit_pss(jt)
                        for jt in range(njt):
                            if jt + 2 < njt:
                                pts[jt + 2] = emit_pss(jt + 2)
                            pt = pts.pop(jt)
                            nc.tensor.matmul(
                                pso[:],
                                vsb[:, jt, h * HD:(h + 1) * HD],
                                pt[:],
                                start=(jt == 0),
                                stop=(jt == njt - 1),
                            )
                            nc.tensor.matmul(
                                psd[:],
                                ones[:, 0:1],
                                pt[:],
                                start=(jt == 0),
                                stop=(jt == njt - 1),
                            )
                        # 1/d via single-op approx reciprocal (~18 bits, way
                        # beyond the ~12-bit fp32r pipeline); exact reciprocal
                        # costs 3.3us and Ln/Exp thrash the ACT table
                        drc = sml.tile([1, TCH], F32, tag="drc")
                        nc.vector.reciprocal_approx_fast(drc[:], psd[:])
                        bc = sml.tile([128, TCH], F32, tag="bc")
                        nc.gpsimd.partition_broadcast(bc[:], drc[:])
                        nc.vector.tensor_mul(otc[:, h, :], pso[:], bc[:])
                    # wo projection for the 4 t-tiles of this chunk
                    for lt in range(4):
                        for oc in range(NCH):
                            ps = ps2.tile([128, TCH], F32, tag="ps_o")
                            for h2 in range(NH):
                                nc.tensor.matmul(
                                    ps[:],
                                    otc[:, h2, lt * 128:(lt + 1) * 128],
                                    wosb[h2][:, oc * TCH:(oc + 1) * TCH],
                                    start=(h2 == 0),
                                    stop=(h2 == NH - 1),
                                )
                            ost = outp.tile([128, TCH], F32, tag="ost")
                            nc.vector.tensor_copy(ost[:], ps[:])
                            nc.sync.dma_start(
                                out[(4 * c + lt) * 128:(4 * c + lt + 1) * 128,
                                    oc * TCH:(oc + 1) * TCH],
                                ost[:],
                            )
    nc.compile()
    return nc


def _get_built(causal: bool):
    if causal not in _BUILT:
        _BUILT[causal] = _build(causal)
    return _BUILT[causal]


def _diag_masks():
    # masks[q][jl, ii] = 1 if key (128*q + jl) <= query ii within the chunk
    q = np.arange(4)[:, None, None]
    jl = np.arange(128)[None, :, None]
    ii = np.arange(TCH)[None, None, :]
    return (ii >= 128 * q + jl).astype(np.float32)


def kernel(x, mask, wqkv, wo):
    global LAST_RESULTS
    from concourse.bass_utils import run_bass_kernel_spmd

    x = np.ascontiguousarray(np.asarray(x, dtype=np.float32))
    wqkv = np.ascontiguousarray(np.asarray(wqkv, dtype=np.float32))
    wo_f = np.ascontiguousarray(np.asarray(wo, dtype=np.float32))
    mask_np = np.asarray(mask).reshape(T, T).astype(bool)
    causal = bool(np.array_equal(mask_np, np.tril(np.ones((T, T), dtype=bool))))

    nc = _get_built(causal)
    masks_arr = _diag_masks()
    maskT = None
    if not causal:
        maskT = np.ascontiguousarray(mask_np.T.astype(np.float32))

    in_maps = []
    for core in range(NCORES):
        b, g = divmod(core, NH)
        xT = np.ascontiguousarray(x[b].T)
        wq = wqkv[:, 0 * H * HD + g * NH * HD:0 * H * HD + (g + 1) * NH * HD]
        wk = wqkv[:, 1 * H * HD + g * NH * HD:1 * H * HD + (g + 1) * NH * HD]
        wv = wqkv[:, 2 * H * HD + g * NH * HD:2 * H * HD + (g + 1) * NH * HD]
        wqkv_g = np.ascontiguousarray(np.concatenate([wq, wk, wv], axis=1))
        wo_g = np.ascontiguousarray(wo_f[g * NH * HD:(g + 1) * NH * HD, :])
        m = {"xT": xT, "wqkv": wqkv_g, "wo": wo_g, "masks": masks_arr}
        if maskT is not None:
            m["maskT"] = maskT
        in_maps.append(m)

    trace = bool(os.environ.get("ATTN_TRACE"))
    res = run_bass_kernel_spmd(nc, in_maps, core_ids=list(range(NCORES)),
                               trace=trace)
    LAST_RESULTS = res

    acc = np.zeros((B, T, D), dtype=np.float64)
    for core in range(NCORES):
        b = core // NH
        acc[b] += res.results[core]["out"].astype(np.float64)
    return acc.astype(np.float32)


# revision 16
# speedup vs baseline: 1.4929x; 1.0842x over previous
"""Causal multi-head attention (B=2, T=2048, DIM=2048, H=16, HD=128) on 8
Trainium2 NeuronCores.

Sharding: core = 4*b + g  (b = batch 0..1, g = head-group 0..3, 4 heads each).
Each core computes, for its batch b and heads 4g..4g+3:
  QKV projection -> causal attention -> partial out = attn_out @ wo[rows of g]
The host sums the 4 partial outputs per batch (the "all-reduce after wo").

On-device layout avoids every transpose:
  - host passes x[b].T, so projections contract d with d on partitions
  - Q^T/K^T kept as [hd, t] (head dim on partitions)
  - scores computed as S^T = K^T_tile.T @ Q^T  ([j, i] layout)
  - exp via ScalarE; causal masking = multiply diagonal tiles by 0/1 masks
  - P@V computed as O^T via lhsT = V tile (natural [t, hd] layout)
  - denominator via ones-vector matmul, normalization via GPSIMD
    partition-broadcast of 1/d + VectorE multiply
  - wo projection consumes O^T tiles directly as stationary operands
Matmuls run as float32r (full PE rate; ~2-7e-4 rel err end to end).
"""

import math
import os

import numpy as np

B, T, D, H, HD = 2, 2048, 2048, 16, 128
NH = 4            # heads per core
NCORES = 8
TCH = 512         # query-chunk width (moving-operand free size)
NDT = D // 128    # 16 d-tiles (contraction tiles for projections)
NTT = T // 128    # 16 t-tiles
NCH = T // TCH    # 4 query chunks
DQ = 4            # d-tiles per accumulation group (PSUM chain length)
NQ = NDT // DQ    # 4 groups

_BUILT = {}
LAST_RESULTS = None  # BassKernelResults of the most recent kernel() call


def _build(causal: bool):
    import concourse.mybir as mybir
    import concourse.tile as tile
    from concourse import bacc

    F32 = mybir.dt.float32
    F32R = mybir.dt.float32r
    EXP = mybir.ActivationFunctionType.Exp
    scale = 1.0 / math.sqrt(HD)

    nc = bacc.Bacc(None, name="attn")
    xT = nc.dram_tensor("xT", [D, T], F32R, kind="ExternalInput")
    wqkv = nc.dram_tensor("wqkv", [D, 3 * NH * HD], F32R, kind="ExternalInput")
    wo = nc.dram_tensor("wo", [NH * HD, D], F32R, kind="ExternalInput")
    masks = nc.dram_tensor("masks", [4, 128, TCH], F32R, kind="ExternalInput")
    if not causal:
        maskT = nc.dram_tensor("maskT", [T, T], F32R, kind="ExternalInput")
    out = nc.dram_tensor("out", [T, D], F32, kind="ExternalOutput")

    with tile.TileContext(nc) as tc:
        with (
            tc.tile_pool(name="persist", bufs=1) as persist,
            tc.tile_pool(name="ps3", bufs=3, space="PSUM") as ps3,
            tc.tile_pool(name="pso4", bufs=4, space="PSUM") as pso4,
            tc.tile_pool(name="ps1", bufs=1, space="PSUM") as ps1,
        ):
            # persistent operands for the attention phase
            qkt = persist.tile([128, 8, T], F32R)          # slots 0-3: Q^T heads, 4-7: K^T heads
            vsb = persist.tile([128, NTT, NH * HD], F32R)  # V, [t-tile][local t, head*hd]
            msb = persist.tile([128, 4, TCH], F32R)   # diagonal causal masks
            nc.sync.dma_start(msb[:], masks.rearrange("q p i -> p q i"))
            ones_f = persist.tile([128, 1], F32)
            ones = persist.tile([128, 1], F32R)
            nc.vector.memset(ones_f[:], 1.0)
            nc.vector.tensor_copy(ones[:], ones_f[:])

            # ---- Phase A: QKV projections, streaming x^T / wqkv d-tiles ----
            with tc.tile_pool(name="xw", bufs=7) as xw:
                for qg in range(NQ):
                    xts, wqks, wvs = [], [], []
                    for k in range(DQ):
                        di = qg * DQ + k
                        xt_t = xw.tile([128, T], F32R, tag="xt")
                        nc.sync.dma_start(xt_t[:], xT[di * 128:(di + 1) * 128, :])
                        wqk_t = xw.tile([128, 2 * NH * HD], F32R, tag="wqk")
                        nc.sync.dma_start(wqk_t[:],
                                          wqkv[di * 128:(di + 1) * 128, 0:2 * NH * HD])
                        wv_t = xw.tile([128, NH * HD], F32R, tag="wv")
                        nc.sync.dma_start(wv_t[:],
                                          wqkv[di * 128:(di + 1) * 128,
                                               2 * NH * HD:3 * NH * HD])
                        xts.append(xt_t)
                        wqks.append(wqk_t)
                        wvs.append(wv_t)
                    # V first: attention's PV chains need V earliest
                    for tt in range(NTT):
                        ps = ps3.tile([128, NH * HD], F32, tag="ps_s")
                        for k in range(DQ):
                            nc.tensor.matmul(
                                ps[:],
                                xts[k][:, tt * 128:(tt + 1) * 128],
                                wvs[k][:],
                                start=(k == 0),
                                stop=(k == DQ - 1),
                            )
                        dst = vsb[:, tt, :]
                        if qg == 0:
                            nc.scalar.copy(dst, ps[:])
                        else:
                            nc.vector.tensor_add(dst, dst, ps[:])
                    # Q^T / K^T, ordered so K0/Q0 chunk 0 completes first
                    for tch in range(NCH):
                        for s in (4, 0, 5, 1, 6, 2, 7, 3):
                            ps = ps3.tile([128, TCH], F32, tag="ps_s")
                            for k in range(DQ):
                                nc.tensor.matmul(
                                    ps[:],
                                    wqks[k][:, s * 128:(s + 1) * 128],
                                    xts[k][:, tch * TCH:(tch + 1) * TCH],
                                    start=(k == 0),
                                    stop=(k == DQ - 1),
                                )
                            dst = qkt[:, s, tch * TCH:(tch + 1) * TCH]
                            if qg == 0:
                                nc.scalar.copy(dst, ps[:])
                            else:
                                nc.vector.tensor_add(dst, dst, ps[:])

            # Phase B/C pools open after the stream pool closed, reusing its
            # SBUF region. wo loads land in that freed space.
            with (
                tc.tile_pool(name="post", bufs=1) as post,
                tc.tile_pool(name="work", bufs=4) as work,
                tc.tile_pool(name="sml", bufs=2) as sml,
                tc.tile_pool(name="otp", bufs=2) as otp,
                tc.tile_pool(name="outp", bufs=2) as outp,
            ):
                # separate 2D tiles: a DMA into a 3D middle-index slice
                # ([128, 1, D] dst AP) hard-faults the exec unit
                wosb = []
                for et in range(NH):
                    wt_ = post.tile([128, D], F32R, tag=f"wos{et}")
                    nc.sync.dma_start(wt_[:], wo[et * 128:(et + 1) * 128, :])
                    wosb.append(wt_)

                # ---- Phase B+C: attention per chunk, with the previous
                # chunk's wo-projection chains interleaved into the jt loop so
                # the in-order PE never sits on PSUM slot recycling ----
                def emit_pc_chain(c0, lt, oc, otc0):
                    ps = pso4.tile([128, TCH], F32, tag="ps_o")
                    for h2 in range(NH):
                        nc.tensor.matmul(
                            ps[:],
                            otc0[:, h2, lt * 128:(lt + 1) * 128],
                            wosb[h2][:, oc * TCH:(oc + 1) * TCH],
                            start=(h2 == 0),
                            stop=(h2 == NH - 1),
                        )
                    ost = outp.tile([128, TCH], F32, tag="ost")
                    nc.vector.tensor_copy(ost[:], ps[:])
                    nc.sync.dma_start(
                        out[(4 * c0 + lt) * 128:(4 * c0 + lt + 1) * 128,
                            oc * TCH:(oc + 1) * TCH],
                        ost[:],
                    )

                pending = []
                for c in range(NCH):
                    otc = otp.tile([128, NH, TCH], F32R, tag="ot")
                    njt = 4 * (c + 1) if causal else NTT
                    steps_total = NH * njt
                    spacing = max(1, steps_total // len(pending)) if pending else 0
                    step = 0
                    for h in range(NH):
                        pso = pso4.tile([128, TCH], F32, tag="ps_o")
                        psd = ps1.tile([1, TCH], F32, tag="ps_d")

                        def emit_pss(jt):
                            pss = ps3.tile([128, TCH], F32, tag="ps_s")
                            nc.tensor.matmul(
                                pss[:],
                                qkt[:, 4 + h, jt * 128:(jt + 1) * 128],
                                qkt[:, h, c * TCH:(c + 1) * TCH],
                                start=True,
                                stop=True,
                            )
                            pt = work.tile([128, TCH], F32R, tag="pt")
                            nc.scalar.activation(pt[:], pss[:], EXP, scale=scale)
                            if causal:
                                qd = jt - 4 * c
                                if qd >= 0:
                                    nc.vector.tensor_mul(pt[:], pt[:], msb[:, qd, :])
                            else:
                                mt = work.tile([128, TCH], F32R, tag="mt")
                                nc.sync.dma_start(
                                    mt[:],
                                    maskT[jt * 128:(jt + 1) * 128,
                                          c * TCH:(c + 1) * TCH],
                                )
                                nc.vector.tensor_mul(pt[:], pt[:], mt[:])
                            return pt

                        pts = {}
                        for jt in range(min(2, njt)):
                            pts[jt] = emit_pss(jt)
                        for jt in range(njt):
                            if jt + 2 < njt:
                                pts[jt + 2] = emit_pss(jt + 2)
                            pt = pts.pop(jt)
                            nc.tensor.matmul(
                                pso[:],
                                vsb[:, jt, h * HD:(h + 1) * HD],
                                pt[:],
                                start=(jt == 0),
                                stop=(jt == njt - 1),
                            )
                            nc.tensor.matmul(
                                psd[:],
                                ones[:, 0:1],
                                pt[:],
                                start=(jt == 0),
                                stop=(jt == njt - 1),
                            )
                            step += 1
                            if pending and spacing and step % spacing == 0:
                                emit_pc_chain(*pending.pop(0))
                        # 1/d via single-op approx reciprocal (~18 bits, way
                        # beyond the ~12-bit fp32r pipeline); exact reciprocal
                        # costs 3.3us and Ln/Exp thrash the ACT table
                        drc = sml.tile([1, TCH], F32, tag="drc")
                        nc.vector.reciprocal_approx_fast(drc[:], psd[:])
                        bc = sml.tile([128, TCH], F32, tag="bc")
                        nc.gpsimd.partition_broadcast(bc[:], drc[:])
                        nc.vector.tensor_mul(otc[:, h, :], pso[:], bc[:])
                    while pending:
                        emit_pc_chain(*pending.pop(0))
                    pending = [(c, lt, oc, otc)
                               for lt in range(4) for oc in range(NCH)]
                for chain in pending:
                    emit_pc_chain(*chain)
    nc.compile()
    return nc


def _get_built(causal: bool):
    if causal not in _BUILT:
        _BUILT[causal] = _build(causal)
    return _BUILT[causal]


def _diag_masks():
    # masks[q][jl, ii] = 1 if key (128*q + jl) <= query ii within the chunk
    q = np.arange(4)[:, None, None]
    jl = np.arange(128)[None, :, None]
    ii = np.arange(TCH)[None, None, :]
    return (ii >= 128 * q + jl).astype(np.float32)


def kernel(x, mask, wqkv, wo):
    global LAST_RESULTS
    from concourse.bass_utils import run_bass_kernel_spmd

    x = np.ascontiguousarray(np.asarray(x, dtype=np.float32))
    wqkv = np.ascontiguousarray(np.asarray(wqkv, dtype=np.float32))
    wo_f = np.ascontiguousarray(np.asarray(wo, dtype=np.float32))
    mask_np = np.asarray(mask).reshape(T, T).astype(bool)
    causal = bool(np.array_equal(mask_np, np.tril(np.ones((T, T), dtype=bool))))

    nc = _get_built(causal)
    masks_arr = _diag_masks()
    maskT = None
    if not causal:
        maskT = np.ascontiguousarray(mask_np.T.astype(np.float32))

    in_maps = []
    for core in range(NCORES):
        b, g = divmod(core, NH)
        xT = np.ascontiguousarray(x[b].T)
        wq = wqkv[:, 0 * H * HD + g * NH * HD:0 * H * HD + (g + 1) * NH * HD]
        wk = wqkv[:, 1 * H * HD + g * NH * HD:1 * H * HD + (g + 1) * NH * HD]
        wv = wqkv[:, 2 * H * HD + g * NH * HD:2 * H * HD + (g + 1) * NH * HD]
        wqkv_g = np.ascontiguousarray(np.concatenate([wq, wk, wv], axis=1))
        wo_g = np.ascontiguousarray(wo_f[g * NH * HD:(g + 1) * NH * HD, :])
        m = {"xT": xT, "wqkv": wqkv_g, "wo": wo_g, "masks": masks_arr}
        if maskT is not None:
            m["maskT"] = maskT
        in_maps.append(m)

    trace = bool(os.environ.get("ATTN_TRACE"))
    res = run_bass_kernel_spmd(nc, in_maps, core_ids=list(range(NCORES)),
                               trace=trace)
    LAST_RESULTS = res

    acc = np.zeros((B, T, D), dtype=np.float64)
    for core in range(NCORES):
        b = core // NH
        acc[b] += res.results[core]["out"].astype(np.float64)
    return acc.astype(np.float32)


# revision 20
# speedup vs baseline: 1.5354x; 1.0285x over previous
"""Causal multi-head attention (B=2, T=2048, DIM=2048, H=16, HD=128) on 8
Trainium2 NeuronCores.

Sharding: core = 4*b + g  (b = batch 0..1, g = head-group 0..3, 4 heads each).
Each core computes, for its batch b and heads 4g..4g+3:
  QKV projection -> causal attention -> partial out = attn_out @ wo[rows of g]
The host sums the 4 partial outputs per batch (the "all-reduce after wo").

On-device layout avoids every transpose:
  - host passes x[b].T, so projections contract d with d on partitions
  - Q^T/K^T kept as [hd, t] (head dim on partitions)
  - scores computed as S^T = K^T_tile.T @ Q^T  ([j, i] layout)
  - exp via ScalarE; causal masking = multiply diagonal tiles by 0/1 masks
  - P@V computed as O^T via lhsT = V tile (natural [t, hd] layout)
  - denominator via ones-vector matmul, normalization via GPSIMD
    partition-broadcast of 1/d + VectorE multiply
  - wo projection consumes O^T tiles directly as stationary operands
Matmuls run as float32r (full PE rate; ~2-7e-4 rel err end to end).
"""

import math
import os

import numpy as np

B, T, D, H, HD = 2, 2048, 2048, 16, 128
NH = 4            # heads per core
NCORES = 8
TCH = 512         # query-chunk width (moving-operand free size)
NDT = D // 128    # 16 d-tiles (contraction tiles for projections)
NTT = T // 128    # 16 t-tiles
NCH = T // TCH    # 4 query chunks
DQ = 4            # d-tiles per accumulation group (PSUM chain length)
NQ = NDT // DQ    # 4 groups

_BUILT = {}
LAST_RESULTS = None  # BassKernelResults of the most recent kernel() call


def _build(causal: bool):
    import concourse.mybir as mybir
    import concourse.tile as tile
    from concourse import bacc

    F32 = mybir.dt.float32
    F32R = mybir.dt.float32r
    EXP = mybir.ActivationFunctionType.Exp
    scale = 1.0 / math.sqrt(HD)

    nc = bacc.Bacc(None, name="attn")
    xT = nc.dram_tensor("xT", [D, T], F32R, kind="ExternalInput")
    wqkv = nc.dram_tensor("wqkv", [D, 3 * NH * HD], F32R, kind="ExternalInput")
    wo = nc.dram_tensor("wo", [NH * HD, D], F32R, kind="ExternalInput")
    masks = nc.dram_tensor("masks", [4, 128, TCH], F32R, kind="ExternalInput")
    if not causal:
        maskT = nc.dram_tensor("maskT", [T, T], F32R, kind="ExternalInput")
    out = nc.dram_tensor("out", [T, D], F32, kind="ExternalOutput")

    with tile.TileContext(nc) as tc:
        with (
            tc.tile_pool(name="persist", bufs=1) as persist,
            tc.tile_pool(name="ps3", bufs=3, space="PSUM") as ps3,
            tc.tile_pool(name="pso4", bufs=4, space="PSUM") as pso4,
            tc.tile_pool(name="ps1", bufs=1, space="PSUM") as ps1,
        ):
            # persistent operands for the attention phase
            qkt = persist.tile([128, 8, T], F32R)          # slots 0-3: Q^T heads, 4-7: K^T heads
            vsb = persist.tile([128, NTT, NH * HD], F32R)  # V, [t-tile][local t, head*hd]
            msb = persist.tile([128, 4, TCH], F32R)   # diagonal causal masks
            nc.sync.dma_start(msb[:], masks.rearrange("q p i -> p q i"))
            ones_f = persist.tile([128, 1], F32)
            ones = persist.tile([128, 1], F32R)
            nc.vector.memset(ones_f[:], 1.0)
            nc.vector.tensor_copy(ones[:], ones_f[:])
            # dummy broadcast: preload the GpSimd PartitionBroadcast ucode
            # library now (~11us HBM fetch) so the first real normalize
            # doesn't stall the whole attention pipeline on LIBRARY_RELOAD
            warm = persist.tile([128, 1], F32)
            nc.gpsimd.partition_broadcast(warm[:], ones_f[0:1, :])

            # ---- Phase A: QKV projections, streaming x^T / wqkv d-tiles ----
            with tc.tile_pool(name="xw", bufs=7) as xw:
                for qg in range(NQ):
                    xts, wqks, wvs = [], [], []
                    for k in range(DQ):
                        di = qg * DQ + k
                        xt_t = xw.tile([128, T], F32R, tag="xt")
                        nc.sync.dma_start(xt_t[:], xT[di * 128:(di + 1) * 128, :])
                        wqk_t = xw.tile([128, 2 * NH * HD], F32R, tag="wqk")
                        nc.sync.dma_start(wqk_t[:],
                                          wqkv[di * 128:(di + 1) * 128, 0:2 * NH * HD])
                        wv_t = xw.tile([128, NH * HD], F32R, tag="wv")
                        nc.sync.dma_start(wv_t[:],
                                          wqkv[di * 128:(di + 1) * 128,
                                               2 * NH * HD:3 * NH * HD])
                        xts.append(xt_t)
                        wqks.append(wqk_t)
                        wvs.append(wv_t)
                    # V first: attention's PV chains need V earliest
                    for tt in range(NTT):
                        ps = ps3.tile([128, NH * HD], F32, tag="ps_s")
                        for k in range(DQ):
                            nc.tensor.matmul(
                                ps[:],
                                xts[k][:, tt * 128:(tt + 1) * 128],
                                wvs[k][:],
                                start=(k == 0),
                                stop=(k == DQ - 1),
                            )
                        dst = vsb[:, tt, :]
                        if qg == 0:
                            nc.scalar.copy(dst, ps[:])
                        else:
                            nc.vector.tensor_add(dst, dst, ps[:])
                    # Q^T / K^T, ordered so K0/Q0 chunk 0 completes first
                    for tch in range(NCH):
                        for s in (4, 0, 5, 1, 6, 2, 7, 3):
                            ps = ps3.tile([128, TCH], F32, tag="ps_s")
                            for k in range(DQ):
                                nc.tensor.matmul(
                                    ps[:],
                                    wqks[k][:, s * 128:(s + 1) * 128],
                                    xts[k][:, tch * TCH:(tch + 1) * TCH],
                                    start=(k == 0),
                                    stop=(k == DQ - 1),
                                )
                            dst = qkt[:, s, tch * TCH:(tch + 1) * TCH]
                            if qg == 0:
                                nc.scalar.copy(dst, ps[:])
                            else:
                                nc.vector.tensor_add(dst, dst, ps[:])

            # Phase B/C pools open after the stream pool closed, reusing its
            # SBUF region. wo loads land in that freed space.
            with (
                tc.tile_pool(name="post", bufs=1) as post,
                tc.tile_pool(name="work", bufs=4) as work,
                tc.tile_pool(name="sml", bufs=2) as sml,
                tc.tile_pool(name="otp", bufs=2) as otp,
                tc.tile_pool(name="outp", bufs=2) as outp,
            ):
                # separate 2D tiles: a DMA into a 3D middle-index slice
                # ([128, 1, D] dst AP) hard-faults the exec unit
                wosb = []
                for et in range(NH):
                    wt_ = post.tile([128, D], F32R, tag=f"wos{et}")
                    nc.sync.dma_start(wt_[:], wo[et * 128:(et + 1) * 128, :])
                    wosb.append(wt_)

                # ---- Phase B+C: attention per chunk, with the previous
                # chunk's wo-projection chains interleaved into the jt loop so
                # the in-order PE never sits on PSUM slot recycling ----
                pc_n = [0]

                def emit_pc_chain(c0, lt, oc, otc0, alternate=False):
                    ps = pso4.tile([128, TCH], F32, tag="ps_o")
                    for h2 in range(NH):
                        nc.tensor.matmul(
                            ps[:],
                            otc0[:, h2, lt * 128:(lt + 1) * 128],
                            wosb[h2][:, oc * TCH:(oc + 1) * TCH],
                            start=(h2 == 0),
                            stop=(h2 == NH - 1),
                        )
                    ost = outp.tile([128, TCH], F32, tag="ost")
                    pc_n[0] += 1
                    if alternate and pc_n[0] % 2 == 0:
                        nc.scalar.copy(ost[:], ps[:])
                    else:
                        nc.vector.tensor_copy(ost[:], ps[:])
                    nc.sync.dma_start(
                        out[(4 * c0 + lt) * 128:(4 * c0 + lt + 1) * 128,
                            oc * TCH:(oc + 1) * TCH],
                        ost[:],
                    )

                pending = []
                for c in range(NCH):
                    otc = otp.tile([128, NH, TCH], F32R, tag="ot")
                    njt = 4 * (c + 1) if causal else NTT
                    steps_total = NH * njt
                    spacing = max(1, steps_total // len(pending)) if pending else 0
                    step = 0
                    for h in range(NH):
                        pso = pso4.tile([128, TCH], F32, tag="ps_o")
                        psd = ps1.tile([1, TCH], F32, tag="ps_d")

                        def emit_pss(jt):
                            pss = ps3.tile([128, TCH], F32, tag="ps_s")
                            nc.tensor.matmul(
                                pss[:],
                                qkt[:, 4 + h, jt * 128:(jt + 1) * 128],
                                qkt[:, h, c * TCH:(c + 1) * TCH],
                                start=True,
                                stop=True,
                            )
                            pt = work.tile([128, TCH], F32R, tag="pt")
                            nc.scalar.activation(pt[:], pss[:], EXP, scale=scale)
                            if causal:
                                qd = jt - 4 * c
                                if qd >= 0:
                                    nc.vector.tensor_mul(pt[:], pt[:], msb[:, qd, :])
                            else:
                                mt = work.tile([128, TCH], F32R, tag="mt")
                                nc.sync.dma_start(
                                    mt[:],
                                    maskT[jt * 128:(jt + 1) * 128,
                                          c * TCH:(c + 1) * TCH],
                                )
                                nc.vector.tensor_mul(pt[:], pt[:], mt[:])
                            return pt

                        pts = {}
                        for jt in range(min(2, njt)):
                            pts[jt] = emit_pss(jt)
                        for jt in range(njt):
                            if jt + 2 < njt:
                                pts[jt + 2] = emit_pss(jt + 2)
                            pt = pts.pop(jt)
                            nc.tensor.matmul(
                                pso[:],
                                vsb[:, jt, h * HD:(h + 1) * HD],
                                pt[:],
                                start=(jt == 0),
                                stop=(jt == njt - 1),
                            )
                            nc.tensor.matmul(
                                psd[:],
                                ones[:, 0:1],
                                pt[:],
                                start=(jt == 0),
                                stop=(jt == njt - 1),
                            )
                            step += 1
                            if pending and spacing and step % spacing == 0:
                                emit_pc_chain(*pending.pop(0))
                        # 1/d via single-op approx reciprocal (~18 bits, way
                        # beyond the ~12-bit fp32r pipeline); exact reciprocal
                        # costs 3.3us and Ln/Exp thrash the ACT table
                        drc = sml.tile([1, TCH], F32, tag="drc")
                        nc.vector.reciprocal_approx_fast(drc[:], psd[:])
                        bc = sml.tile([128, TCH], F32, tag="bc")
                        nc.gpsimd.partition_broadcast(bc[:], drc[:])
                        nc.vector.tensor_mul(otc[:, h, :], pso[:], bc[:])
                    while pending:
                        emit_pc_chain(*pending.pop(0))
                    pending = [(c, lt, oc, otc)
                               for lt in range(4) for oc in range(NCH)]
                # tail drain: alternate the PSUM->SBUF copies between DVE and
                # ScalarE so slot recycling isn't single-engine-latency-bound
                for chain in pending:
                    emit_pc_chain(*chain, alternate=True)
    nc.compile()
    return nc


def _get_built(causal: bool):
    if causal not in _BUILT:
        _BUILT[causal] = _build(causal)
    return _BUILT[causal]


def _diag_masks():
    # masks[q][jl, ii] = 1 if key (128*q + jl) <= query ii within the chunk
    q = np.arange(4)[:, None, None]
    jl = np.arange(128)[None, :, None]
    ii = np.arange(TCH)[None, None, :]
    return (ii >= 128 * q + jl).astype(np.float32)


def kernel(x, mask, wqkv, wo):
    global LAST_RESULTS
    from concourse.bass_utils import run_bass_kernel_spmd

    x = np.ascontiguousarray(np.asarray(x, dtype=np.float32))
    wqkv = np.ascontiguousarray(np.asarray(wqkv, dtype=np.float32))
    wo_f = np.ascontiguousarray(np.asarray(wo, dtype=np.float32))
    mask_np = np.asarray(mask).reshape(T, T).astype(bool)
    causal = bool(np.array_equal(mask_np, np.tril(np.ones((T, T), dtype=bool))))

    nc = _get_built(causal)
    masks_arr = _diag_masks()
    maskT = None
    if not causal:
        maskT = np.ascontiguousarray(mask_np.T.astype(np.float32))

    in_maps = []
    for core in range(NCORES):
        b, g = divmod(core, NH)
        xT = np.ascontiguousarray(x[b].T)
        wq = wqkv[:, 0 * H * HD + g * NH * HD:0 * H * HD + (g + 1) * NH * HD]
        wk = wqkv[:, 1 * H * HD + g * NH * HD:1 * H * HD + (g + 1) * NH * HD]
        wv = wqkv[:, 2 * H * HD + g * NH * HD:2 * H * HD + (g + 1) * NH * HD]
        wqkv_g = np.ascontiguousarray(np.concatenate([wq, wk, wv], axis=1))
        wo_g = np.ascontiguousarray(wo_f[g * NH * HD:(g + 1) * NH * HD, :])
        m = {"xT": xT, "wqkv": wqkv_g, "wo": wo_g, "masks": masks_arr}
        if maskT is not None:
            m["maskT"] = maskT
        in_maps.append(m)

    trace = bool(os.environ.get("ATTN_TRACE"))
    res = run_bass_kernel_spmd(nc, in_maps, core_ids=list(range(NCORES)),
                               trace=trace)
    LAST_RESULTS = res

    acc = np.zeros((B, T, D), dtype=np.float64)
    for core in range(NCORES):
        b = core // NH
        acc[b] += res.results[core]["out"].astype(np.float64)
    return acc.astype(np.float32)


# revision 22
# speedup vs baseline: 1.5770x; 1.0271x over previous
"""Causal multi-head attention (B=2, T=2048, DIM=2048, H=16, HD=128) on 8
Trainium2 NeuronCores.

Sharding: core = 4*b + g  (b = batch 0..1, g = head-group 0..3, 4 heads each).
Each core computes, for its batch b and heads 4g..4g+3:
  QKV projection -> causal attention -> partial out = attn_out @ wo[rows of g]
The host sums the 4 partial outputs per batch (the "all-reduce after wo").

On-device layout avoids every transpose:
  - host passes x[b].T, so projections contract d with d on partitions
  - Q^T/K^T kept as [hd, t] (head dim on partitions)
  - scores computed as S^T = K^T_tile.T @ Q^T  ([j, i] layout)
  - exp via ScalarE; causal masking = multiply diagonal tiles by 0/1 masks
  - P@V computed as O^T via lhsT = V tile (natural [t, hd] layout)
  - denominator via ones-vector matmul, normalization via GPSIMD
    partition-broadcast of 1/d + VectorE multiply
  - wo projection consumes O^T tiles directly as stationary operands
Matmuls run as float32r (full PE rate; ~2-7e-4 rel err end to end).
"""

import math
import os

import numpy as np

B, T, D, H, HD = 2, 2048, 2048, 16, 128
NH = 4            # heads per core
NCORES = 8
TCH = 512         # query-chunk width (moving-operand free size)
NDT = D // 128    # 16 d-tiles (contraction tiles for projections)
NTT = T // 128    # 16 t-tiles
NCH = T // TCH    # 4 query chunks
DQ = 4            # d-tiles per accumulation group (PSUM chain length)
NQ = NDT // DQ    # 4 groups

_BUILT = {}
LAST_RESULTS = None  # BassKernelResults of the most recent kernel() call


def _build(causal: bool):
    import concourse.mybir as mybir
    import concourse.tile as tile
    from concourse import bacc

    F32 = mybir.dt.float32
    F32R = mybir.dt.float32r
    EXP = mybir.ActivationFunctionType.Exp
    scale = 1.0 / math.sqrt(HD)

    nc = bacc.Bacc(None, name="attn")
    xT = nc.dram_tensor("xT", [D, T], F32R, kind="ExternalInput")
    wqkv = nc.dram_tensor("wqkv", [D, 3 * NH * HD], F32R, kind="ExternalInput")
    wo = nc.dram_tensor("wo", [NH * HD, D], F32R, kind="ExternalInput")
    masks = nc.dram_tensor("masks", [128, 4 * TCH], F32R, kind="ExternalInput")
    if not causal:
        maskT = nc.dram_tensor("maskT", [T, T], F32R, kind="ExternalInput")
    out = nc.dram_tensor("out", [T, D], F32, kind="ExternalOutput")

    with tile.TileContext(nc) as tc:
        with (
            tc.tile_pool(name="persist", bufs=1) as persist,
            tc.tile_pool(name="ps3", bufs=3, space="PSUM") as ps3,
            tc.tile_pool(name="pso4", bufs=4, space="PSUM") as pso4,
            tc.tile_pool(name="ps1", bufs=1, space="PSUM") as ps1,
        ):
            # persistent operands for the attention phase
            qkt = persist.tile([128, 8, T], F32R)          # slots 0-3: Q^T heads, 4-7: K^T heads
            vsb = persist.tile([128, NTT, NH * HD], F32R)  # V, [t-tile][local t, head*hd]
            ones_f = persist.tile([128, 1], F32)
            ones = persist.tile([128, 1], F32R)
            nc.vector.memset(ones_f[:], 1.0)
            nc.vector.tensor_copy(ones[:], ones_f[:])
            # dummy broadcast: preload the GpSimd PartitionBroadcast ucode
            # library now (~11us HBM fetch) so the first real normalize
            # doesn't stall the whole attention pipeline on LIBRARY_RELOAD
            warm = persist.tile([128, 1], F32)
            nc.gpsimd.partition_broadcast(warm[:], ones_f[0:1, :])

            # ---- Phase A: QKV projections, streaming x^T / wqkv d-tiles ----
            with tc.tile_pool(name="xw", bufs=8) as xw:
                for qg in range(NQ):
                    xts, wqks, wvs = [], [], []
                    for k in range(DQ):
                        di = qg * DQ + k
                        xt_t = xw.tile([128, T], F32R, tag="xt")
                        nc.sync.dma_start(xt_t[:], xT[di * 128:(di + 1) * 128, :])
                        wqk_t = xw.tile([128, 2 * NH * HD], F32R, tag="wqk")
                        nc.sync.dma_start(wqk_t[:],
                                          wqkv[di * 128:(di + 1) * 128, 0:2 * NH * HD])
                        wv_t = xw.tile([128, NH * HD], F32R, tag="wv", bufs=7)
                        nc.sync.dma_start(wv_t[:],
                                          wqkv[di * 128:(di + 1) * 128,
                                               2 * NH * HD:3 * NH * HD])
                        xts.append(xt_t)
                        wqks.append(wqk_t)
                        wvs.append(wv_t)
                    # V first: attention's PV chains need V earliest
                    for tt in range(NTT):
                        ps = ps3.tile([128, NH * HD], F32, tag="ps_s")
                        for k in range(DQ):
                            nc.tensor.matmul(
                                ps[:],
                                xts[k][:, tt * 128:(tt + 1) * 128],
                                wvs[k][:],
                                start=(k == 0),
                                stop=(k == DQ - 1),
                            )
                        dst = vsb[:, tt, :]
                        if qg == 0:
                            nc.scalar.copy(dst, ps[:])
                        else:
                            nc.vector.tensor_add(dst, dst, ps[:])
                    # Q^T / K^T, ordered so K0/Q0 chunk 0 completes first
                    for tch in range(NCH):
                        for s in (4, 0, 5, 1, 6, 2, 7, 3):
                            ps = ps3.tile([128, TCH], F32, tag="ps_s")
                            for k in range(DQ):
                                nc.tensor.matmul(
                                    ps[:],
                                    wqks[k][:, s * 128:(s + 1) * 128],
                                    xts[k][:, tch * TCH:(tch + 1) * TCH],
                                    start=(k == 0),
                                    stop=(k == DQ - 1),
                                )
                            dst = qkt[:, s, tch * TCH:(tch + 1) * TCH]
                            if qg == 0:
                                nc.scalar.copy(dst, ps[:])
                            else:
                                nc.vector.tensor_add(dst, dst, ps[:])

            # Phase B/C pools open after the stream pool closed, reusing its
            # SBUF region. wo loads land in that freed space.
            with (
                tc.tile_pool(name="post", bufs=1) as post,
                tc.tile_pool(name="work", bufs=4) as work,
                tc.tile_pool(name="sml", bufs=2) as sml,
                tc.tile_pool(name="otp", bufs=2) as otp,
                tc.tile_pool(name="outp", bufs=2) as outp,
            ):
                msb = post.tile([128, 4 * TCH], F32R)  # diagonal causal masks
                nc.sync.dma_start(msb[:], masks[:])
                # separate 2D tiles: a DMA into a 3D middle-index slice
                # ([128, 1, D] dst AP) hard-faults the exec unit
                wosb = []
                for et in range(NH):
                    wt_ = post.tile([128, D], F32R, tag=f"wos{et}")
                    nc.sync.dma_start(wt_[:], wo[et * 128:(et + 1) * 128, :])
                    wosb.append(wt_)

                # ---- Phase B+C: attention per chunk, with the previous
                # chunk's wo-projection chains interleaved into the jt loop so
                # the in-order PE never sits on PSUM slot recycling ----
                pc_n = [0]

                def emit_pc_chain(c0, lt, oc, otc0, alternate=False):
                    if alternate and pc_n[0] % 2 == 0:
                        ps = ps3.tile([128, TCH], F32, tag="ps_s")
                    else:
                        ps = pso4.tile([128, TCH], F32, tag="ps_o")
                    for h2 in range(NH):
                        nc.tensor.matmul(
                            ps[:],
                            otc0[:, h2, lt * 128:(lt + 1) * 128],
                            wosb[h2][:, oc * TCH:(oc + 1) * TCH],
                            start=(h2 == 0),
                            stop=(h2 == NH - 1),
                        )
                    ost = outp.tile([128, TCH], F32, tag="ost")
                    pc_n[0] += 1
                    if alternate and pc_n[0] % 2 == 0:
                        nc.scalar.copy(ost[:], ps[:])
                    else:
                        nc.vector.tensor_copy(ost[:], ps[:])
                    nc.sync.dma_start(
                        out[(4 * c0 + lt) * 128:(4 * c0 + lt + 1) * 128,
                            oc * TCH:(oc + 1) * TCH],
                        ost[:],
                    )

                pending = []
                for c in range(NCH):
                    otc = otp.tile([128, NH, TCH], F32R, tag="ot")
                    njt = 4 * (c + 1) if causal else NTT
                    steps_total = NH * njt
                    spacing = max(1, steps_total // len(pending)) if pending else 0
                    step = 0
                    for h in range(NH):
                        pso = pso4.tile([128, TCH], F32, tag="ps_o")
                        psd = ps1.tile([1, TCH], F32, tag="ps_d")

                        def emit_pss(jt):
                            pss = ps3.tile([128, TCH], F32, tag="ps_s")
                            nc.tensor.matmul(
                                pss[:],
                                qkt[:, 4 + h, jt * 128:(jt + 1) * 128],
                                qkt[:, h, c * TCH:(c + 1) * TCH],
                                start=True,
                                stop=True,
                            )
                            pt = work.tile([128, TCH], F32R, tag="pt")
                            nc.scalar.activation(pt[:], pss[:], EXP, scale=scale)
                            if causal:
                                qd = jt - 4 * c
                                if qd >= 0:
                                    nc.vector.tensor_mul(pt[:], pt[:], msb[:, qd * TCH:(qd + 1) * TCH])
                            else:
                                mt = work.tile([128, TCH], F32R, tag="mt")
                                nc.sync.dma_start(
                                    mt[:],
                                    maskT[jt * 128:(jt + 1) * 128,
                                          c * TCH:(c + 1) * TCH],
                                )
                                nc.vector.tensor_mul(pt[:], pt[:], mt[:])
                            return pt

                        pts = {}
                        for jt in range(min(2, njt)):
                            pts[jt] = emit_pss(jt)
                        for jt in range(njt):
                            if jt + 2 < njt:
                                pts[jt + 2] = emit_pss(jt + 2)
                            pt = pts.pop(jt)
                            nc.tensor.matmul(
                                pso[:],
                                vsb[:, jt, h * HD:(h + 1) * HD],
                                pt[:],
                                start=(jt == 0),
                                stop=(jt == njt - 1),
                            )
                            nc.tensor.matmul(
                                psd[:],
                                ones[:, 0:1],
                                pt[:],
                                start=(jt == 0),
                                stop=(jt == njt - 1),
                            )
                            step += 1
                            if pending and spacing and step % spacing == 0:
                                emit_pc_chain(*pending.pop(0))
                        # 1/d via single-op approx reciprocal (~18 bits, way
                        # beyond the ~12-bit fp32r pipeline); exact reciprocal
                        # costs 3.3us and Ln/Exp thrash the ACT table
                        drc = sml.tile([1, TCH], F32, tag="drc")
                        nc.vector.reciprocal_approx_fast(drc[:], psd[:])
                        bc = sml.tile([128, TCH], F32, tag="bc")
                        nc.gpsimd.partition_broadcast(bc[:], drc[:])
                        nc.vector.tensor_mul(otc[:, h, :], pso[:], bc[:])
                    while pending:
                        emit_pc_chain(*pending.pop(0))
                    pending = [(c, lt, oc, otc)
                               for lt in range(4) for oc in range(NCH)]
                # tail drain: alternate the PSUM->SBUF copies between DVE and
                # ScalarE so slot recycling isn't single-engine-latency-bound
                for chain in pending:
                    emit_pc_chain(*chain, alternate=True)
    nc.compile()
    return nc


def _get_built(causal: bool):
    if causal not in _BUILT:
        _BUILT[causal] = _build(causal)
    return _BUILT[causal]


def _diag_masks():
    # masks[jl, q*TCH + ii] = 1 if key (128*q + jl) <= query ii in the chunk
    q = np.arange(4)[:, None, None]
    jl = np.arange(128)[None, :, None]
    ii = np.arange(TCH)[None, None, :]
    m = (ii >= 128 * q + jl).astype(np.float32)        # [4, 128, TCH]
    return np.ascontiguousarray(m.transpose(1, 0, 2).reshape(128, 4 * TCH))


def kernel(x, mask, wqkv, wo):
    global LAST_RESULTS
    from concourse.bass_utils import run_bass_kernel_spmd

    x = np.ascontiguousarray(np.asarray(x, dtype=np.float32))
    wqkv = np.ascontiguousarray(np.asarray(wqkv, dtype=np.float32))
    wo_f = np.ascontiguousarray(np.asarray(wo, dtype=np.float32))
    mask_np = np.asarray(mask).reshape(T, T).astype(bool)
    causal = bool(np.array_equal(mask_np, np.tril(np.ones((T, T), dtype=bool))))

    nc = _get_built(causal)
    masks_arr = _diag_masks()
    maskT = None
    if not causal:
        maskT = np.ascontiguousarray(mask_np.T.astype(np.float32))

    in_maps = []
    for core in range(NCORES):
        b, g = divmod(core, NH)
        xT = np.ascontiguousarray(x[b].T)
        wq = wqkv[:, 0 * H * HD + g * NH * HD:0 * H * HD + (g + 1) * NH * HD]
        wk = wqkv[:, 1 * H * HD + g * NH * HD:1 * H * HD + (g + 1) * NH * HD]
        wv = wqkv[:, 2 * H * HD + g * NH * HD:2 * H * HD + (g + 1) * NH * HD]
        wqkv_g = np.ascontiguousarray(np.concatenate([wq, wk, wv], axis=1))
        wo_g = np.ascontiguousarray(wo_f[g * NH * HD:(g + 1) * NH * HD, :])
        m = {"xT": xT, "wqkv": wqkv_g, "wo": wo_g, "masks": masks_arr}
        if maskT is not None:
            m["maskT"] = maskT
        in_maps.append(m)

    trace = bool(os.environ.get("ATTN_TRACE"))
    res = run_bass_kernel_spmd(nc, in_maps, core_ids=list(range(NCORES)),
                               trace=trace)
    LAST_RESULTS = res

    acc = np.zeros((B, T, D), dtype=np.float64)
    for core in range(NCORES):
        b = core // NH
        acc[b] += res.results[core]["out"].astype(np.float64)
    return acc.astype(np.float32)
